# revision 21
# baseline (speedup 1.0000x reference)
"""Trainium2 Bass kernel for nn_BilinearPolicy (dense_mlp).

Math (reference):
  ob = trunk_obs(obs)      : [B,256] -> 2048 -> 2048 -> 2048 -> 16384 (ReLU between)
  dl = trunk_dlt(deltas)   : same shapes, different weights
  pred[b,a] = sum_f ob[b, a*512+f] * dl[b, f*32+a]            : [B, 32]

Strategy:
  * Data-parallel over batch: 8 cores x 512 rows, zero collectives.
  * Feature-major activations on chip ([feat(part), batch(free)]), so the
    torch-layout weights [din, dout] are used directly as matmul lhsT tiles
    and no transposes are ever needed. Inputs are transposed on host.
  * bf16 matmuls with fp32 PSUM accumulation. Biases applied during the
    mandatory PSUM->SBUF eviction on the Scalar engine (Relu / Identity).
  * dl's last-layer weight columns are permuted on host from (f,a) to (a,f)
    ordering, so the bilinear diagonal becomes: elementwise multiply of the
    two [16384, 512] outputs, then a segmented 512-row partition reduction.
    The 4 z-tiles of an action are accumulated on the DVE; one one-hot mask
    matmul per action accumulates pred^T [32, 512] in a single PSUM tile.
  * Weight streaming is the supply-critical path (~150 GB/s on one HWDGE
    queue with small lines): weights are pre-grouped on host so each DMA
    moves 4 m-tiles (2 MB, 16 KB/partition lines) and DMAs round-robin
    over the sync + scalar HWDGE queues. Small constants go via gpsimd
    SWDGE so they never queue ahead of weights.
"""

from contextlib import ExitStack

import numpy as np
import ml_dtypes

B, OBS, H, F, A = 4096, 256, 2048, 512, 32
DOUT = F * A            # 16384
NCORES = 8
BPC = B // NCORES       # 512 batch rows per core
P = 128

KT = [2, 16, 16, 16]    # k-tiles per layer
MT = [16, 16, 16, 128]  # m-tiles per layer
GR = [2, 4, 4, 4]       # m-tiles grouped per weight DMA

BF16 = ml_dtypes.bfloat16

# Filled with the BassKernelResults of the most recent run (for test harness).
LAST_RESULTS = None


def _tile_weight(w, G):
    """[D1, D2] fp32 -> [D2/(128G), 128(k), G*D1] bf16. Slice
    [:, (g*Kt + j)*128 : +128] of group tile mtg is the lhsT for
    k-tile j of m-tile mtg*G+g; every partition line is contiguous."""
    d1, d2 = w.shape
    kt, mt = d1 // P, d2 // P
    wt = w.reshape(kt, P, mt, P).transpose(2, 1, 0, 3)      # [mt, k, j, m]
    wt = wt.reshape(mt // G, G, P, kt * P).transpose(0, 2, 1, 3)
    wt = wt.reshape(mt // G, P, G * kt * P)
    return np.ascontiguousarray(wt.astype(BF16))


def _tile_bias(b):
    """[D2] fp32 -> [128, D2/128] fp32; column mt holds bias for m-tile mt
    as a per-partition scalar."""
    return np.ascontiguousarray(b.reshape(-1, P).T.astype(np.float32))


def _build_program():
    import concourse.bass as bass
    import concourse.tile as tile
    from concourse import bacc, mybir
    from concourse.bass import ts

    dt = mybir.dt
    AF = mybir.ActivationFunctionType

    nc = bacc.Bacc(
        "TRN2",
        target_bir_lowering=False,
        debug=False,
        enable_asserts=True,
        num_devices=NCORES,
    )

    def din(name, shape, dtype):
        return nc.dram_tensor(name, shape, dtype, kind="ExternalInput").ap()

    x_d = {
        "o": din("xo", [P, 2 * BPC], dt.bfloat16),
        "d": din("xd", [P, 2 * BPC], dt.bfloat16),
    }
    w_d = {}
    b_d = {}
    for t in ("o", "d"):
        for l in range(4):
            w_d[t, l] = din(f"{t}w{l}", [MT[l] // GR[l], P, GR[l] * KT[l] * P],
                            dt.bfloat16)
            b_d[t, l] = din(f"{t}b{l}", [P, MT[l]], dt.float32)
    masks_d = din("masks", [P, A * A], dt.bfloat16)
    pred_d = nc.dram_tensor("pred", [A, BPC], dt.float32, kind="ExternalOutput").ap()

    with tile.TileContext(nc) as tc, ExitStack() as ctx:
        const = ctx.enter_context(tc.tile_pool(name="const", bufs=1))
        wp = ctx.enter_context(tc.tile_pool(name="wp", bufs=4))
        act = ctx.enter_context(tc.tile_pool(name="act", bufs=1))
        ev = ctx.enter_context(tc.tile_pool(name="ev", bufs=8))
        ps = ctx.enter_context(tc.tile_pool(name="ps", bufs=6, space="PSUM"))
        psp = ctx.enter_context(tc.tile_pool(name="psp", bufs=1, space="PSUM"))

        # weight DMAs round-robin over three HWDGE queues so supply isn't
        # capped by a single queue's descriptor rate
        dma_engs = [nc.sync, nc.scalar]
        rr = [0]

        def wdma(dst, src):
            dma_engs[rr[0] % len(dma_engs)].dma_start(dst, src)
            rr[0] += 1

        # Pre-warm the PE during the initial DMA wait: ~5 us of dummy
        # matmuls lift the HAM clock gate to 8/8 before real work arrives.
        warm = const.tile([P, P], dt.bfloat16, tag="warm")
        nc.vector.memset(warm[:], 0.0)
        warm_ps = psp.tile([P, P], dt.float32, tag="warmps")
        for _ in range(48):
            nc.tensor.matmul(warm_ps[:], warm[:], warm[:], start=True, stop=True)

        # inputs + L0 weights first, in small chunks split across both HWDGE
        # queues in first-use order so the PE can start within a few us;
        # other small constants go on the gpsimd SWDGE queue
        x_sb = {}
        w0_sb = {}
        chunk = GR[0] * KT[0] * P
        for t in ("o", "d"):
            x_sb[t] = const.tile([P, 2, BPC], dt.bfloat16,
                                 tag=f"x{t}", name=f"x{t}")
            for j in range(2):
                wdma(x_sb[t][:, j, :],
                     x_d[t][:, j * BPC:(j + 1) * BPC])
            w0_sb[t] = const.tile([P, (MT[0] // GR[0]) * chunk], dt.bfloat16,
                                  tag=f"w0{t}", name=f"w0{t}")
            for c in range(MT[0] // GR[0]):
                wdma(w0_sb[t][:, c * chunk:(c + 1) * chunk], w_d[t, 0][c])
        bias_sb = {}
        for t in ("o", "d"):
            for l in range(4):
                bias_sb[t, l] = const.tile([P, MT[l]], dt.float32,
                                           tag=f"b{t}{l}", name=f"b{t}{l}")
                nc.gpsimd.dma_start(bias_sb[t, l][:], b_d[t, l][:])
        masks_sb = const.tile([P, A * A], dt.bfloat16, tag="masks")
        nc.gpsimd.dma_start(masks_sb[:], masks_d[:])

        # ---- Trunks: layers 0..2 with ReLU, feature-major throughout.
        # The two trunks are interleaved layer-by-layer so the PE has twice
        # the work per phase start, covering the weight-stream warm-up.
        cur = dict(x_sb)
        for l in range(3):
            for t in ("o", "d"):
                out_t = act.tile([P, MT[l], BPC], dt.bfloat16,
                                 tag=f"h{t}{l % 2}", name=f"h{t}{l}")
                for mtg in range(MT[l] // GR[l]):
                    if l == 0:
                        wt = w0_sb[t]
                    else:
                        wt = wp.tile([P, GR[l] * KT[l] * P], dt.bfloat16,
                                     tag="wbig")
                        wdma(wt[:], w_d[t, l][mtg])
                    for g in range(GR[l]):
                        mt = mtg * GR[l] + g
                        wcol = mt if l == 0 else g  # L0 tile is fully resident
                        pt = ps.tile([P, BPC], dt.float32, tag="mm")
                        for j in range(KT[l]):
                            nc.tensor.matmul(
                                pt[:], wt[:, ts(wcol * KT[l] + j, P)],
                                cur[t][:, j, :],
                                start=(j == 0), stop=(j == KT[l] - 1),
                            )
                        nc.scalar.activation(
                            out_t[:, mt, :], pt[:], AF.Relu,
                            bias=bias_sb[t, l][:, mt:mt + 1],
                        )
                cur[t] = out_t
        h = cur

        # ---- Layer 3 + bilinear diagonal, fused per 128-feature tile.
        pred_ps = psp.tile([A, BPC], dt.float32, tag="pred")
        for a in range(A):  # one weight DMA per trunk covers the whole action
            z_acc = ev.tile([P, BPC], dt.bfloat16, tag="zacc")
            wt = {}
            for t in ("o", "d"):
                wt[t] = wp.tile([P, GR[3] * KT[3] * P], dt.bfloat16,
                                tag="wbig", name=f"w3{t}")
                wdma(wt[t][:], w_d[t, 3][a])
            for g in range(GR[3]):
                mt = a * 4 + g
                s = {}
                for t in ("o", "d"):
                    pt = ps.tile([P, BPC], dt.float32, tag="mm")
                    for j in range(KT[3]):
                        nc.tensor.matmul(
                            pt[:], wt[t][:, ts(g * KT[3] + j, P)],
                            h[t][:, j, :],
                            start=(j == 0), stop=(j == KT[3] - 1),
                        )
                    s[t] = ev.tile([P, BPC], dt.bfloat16, tag="evict",
                                   name=f"s{t}")
                    nc.scalar.activation(
                        s[t][:], pt[:], AF.Identity,
                        bias=bias_sb[t, 3][:, mt:mt + 1],
                    )
                if g == 0:
                    nc.vector.tensor_mul(z_acc[:], s["o"][:], s["d"][:])
                else:
                    zt = ev.tile([P, BPC], dt.bfloat16, tag="ztmp")
                    nc.vector.tensor_mul(zt[:], s["o"][:], s["d"][:])
                    nc.vector.tensor_add(z_acc[:], z_acc[:], zt[:])
            nc.tensor.matmul(
                pred_ps[:], masks_sb[:, ts(a, A)], z_acc[:],
                start=(a == 0), stop=(a == A - 1),
            )

        pred_sb = ev.tile([A, BPC], dt.float32, tag="predsb")
        nc.vector.tensor_copy(pred_sb[:], pred_ps[:])
        nc.sync.dma_start(pred_d[:], pred_sb[:])

    nc.compile()
    return nc


def _prep_inputs(inputs):
    """Host-side layout/dtype prep shared across cores + per-core slices."""
    shared = {}

    for t, pfx in (("o", "obs"), ("d", "dlt")):
        for l in range(4):
            w = np.asarray(inputs[f"{pfx}_W{l}"], np.float32)
            b = np.asarray(inputs[f"{pfx}_b{l}"], np.float32)
            if t == "d" and l == 3:
                # permute columns (f,a) -> (a,f) to match obs layout
                w = w.reshape(H, F, A).transpose(0, 2, 1).reshape(H, DOUT)
                b = b.reshape(F, A).T.reshape(DOUT)
            shared[f"{t}w{l}"] = _tile_weight(w, GR[l])
            shared[f"{t}b{l}"] = _tile_bias(b)

    masks = np.zeros((P, A, A), np.float32)
    for a in range(A):
        masks[:, a, a] = 1.0
    shared["masks"] = np.ascontiguousarray(masks.reshape(P, A * A).astype(BF16))

    obsT = np.asarray(inputs["obs"], np.float32).T.astype(BF16)    # [256, 4096]
    dltT = np.asarray(inputs["deltas"], np.float32).T.astype(BF16)

    in_maps = []
    for c in range(NCORES):
        sl = slice(c * BPC, (c + 1) * BPC)
        m = dict(shared)
        m["xo"] = np.ascontiguousarray(
            obsT[:, sl].reshape(2, P, BPC).transpose(1, 0, 2).reshape(P, 2 * BPC))
        m["xd"] = np.ascontiguousarray(
            dltT[:, sl].reshape(2, P, BPC).transpose(1, 0, 2).reshape(P, 2 * BPC))
        in_maps.append(m)
    return in_maps


_PROGRAM = None


def kernel(**inputs):
    global _PROGRAM, LAST_RESULTS
    from concourse.bass_utils import run_bass_kernel_spmd

    if _PROGRAM is None:
        _PROGRAM = _build_program()
    in_maps = _prep_inputs(inputs)
    res = run_bass_kernel_spmd(_PROGRAM, in_maps, list(range(NCORES)))
    LAST_RESULTS = res
    out = np.empty((B, A), np.float32)
    for c in range(NCORES):
        out[c * BPC:(c + 1) * BPC] = res.results[c]["pred"].T
    return out


# revision 23
# speedup vs baseline: 1.0028x; 1.0028x over previous
"""Trainium2 Bass kernel for nn_BilinearPolicy (dense_mlp).

Math (reference):
  ob = trunk_obs(obs)      : [B,256] -> 2048 -> 2048 -> 2048 -> 16384 (ReLU between)
  dl = trunk_dlt(deltas)   : same shapes, different weights
  pred[b,a] = sum_f ob[b, a*512+f] * dl[b, f*32+a]            : [B, 32]

Strategy:
  * Data-parallel over batch: 8 cores x 512 rows, zero collectives.
  * Feature-major activations on chip ([feat(part), batch(free)]), so the
    torch-layout weights [din, dout] are used directly as matmul lhsT tiles
    and no transposes are ever needed. Inputs are transposed on host.
  * bf16 matmuls with fp32 PSUM accumulation. Biases applied during the
    mandatory PSUM->SBUF eviction on the Scalar engine (Relu / Identity).
  * dl's last-layer weight columns are permuted on host from (f,a) to (a,f)
    ordering, so the bilinear diagonal becomes: elementwise multiply of the
    two [16384, 512] outputs, then a segmented 512-row partition reduction.
    The 4 z-tiles of an action are accumulated on the DVE; one one-hot mask
    matmul per action accumulates pred^T [32, 512] in a single PSUM tile.
  * Weight streaming is the supply-critical path (~150 GB/s on one HWDGE
    queue with small lines): weights are pre-grouped on host so each DMA
    moves 4 m-tiles (2 MB, 16 KB/partition lines) and DMAs round-robin
    over the sync + scalar HWDGE queues. Small constants go via gpsimd
    SWDGE so they never queue ahead of weights.
"""

from contextlib import ExitStack

import numpy as np
import ml_dtypes

B, OBS, H, F, A = 4096, 256, 2048, 512, 32
DOUT = F * A            # 16384
NCORES = 8
BPC = B // NCORES       # 512 batch rows per core
P = 128

KT = [2, 16, 16, 16]    # k-tiles per layer
MT = [16, 16, 16, 128]  # m-tiles per layer
GR = [8, 4, 4, 4]       # m-tiles grouped per weight DMA

BF16 = ml_dtypes.bfloat16

# Filled with the BassKernelResults of the most recent run (for test harness).
LAST_RESULTS = None


def _tile_weight(w, G):
    """[D1, D2] fp32 -> [D2/(128G), 128(k), G*D1] bf16. Slice
    [:, (g*Kt + j)*128 : +128] of group tile mtg is the lhsT for
    k-tile j of m-tile mtg*G+g; every partition line is contiguous."""
    d1, d2 = w.shape
    kt, mt = d1 // P, d2 // P
    wt = w.reshape(kt, P, mt, P).transpose(2, 1, 0, 3)      # [mt, k, j, m]
    wt = wt.reshape(mt // G, G, P, kt * P).transpose(0, 2, 1, 3)
    wt = wt.reshape(mt // G, P, G * kt * P)
    return np.ascontiguousarray(wt.astype(BF16))


def _tile_bias(b):
    """[D2] fp32 -> [128, D2/128] fp32; column mt holds bias for m-tile mt
    as a per-partition scalar."""
    return np.ascontiguousarray(b.reshape(-1, P).T.astype(np.float32))


def _build_program():
    import concourse.bass as bass
    import concourse.tile as tile
    from concourse import bacc, mybir
    from concourse.bass import ts

    dt = mybir.dt
    AF = mybir.ActivationFunctionType

    nc = bacc.Bacc(
        "TRN2",
        target_bir_lowering=False,
        debug=False,
        enable_asserts=True,
        num_devices=NCORES,
    )

    def din(name, shape, dtype):
        return nc.dram_tensor(name, shape, dtype, kind="ExternalInput").ap()

    x_d = {
        "o": din("xo", [P, 2 * BPC], dt.bfloat16),
        "d": din("xd", [P, 2 * BPC], dt.bfloat16),
    }
    w_d = {}
    b_d = {}
    for t in ("o", "d"):
        for l in range(4):
            w_d[t, l] = din(f"{t}w{l}", [MT[l] // GR[l], P, GR[l] * KT[l] * P],
                            dt.bfloat16)
            b_d[t, l] = din(f"{t}b{l}", [P, MT[l]], dt.float32)
    masks_d = din("masks", [P, A * A], dt.bfloat16)
    pred_d = nc.dram_tensor("pred", [A, BPC], dt.float32, kind="ExternalOutput").ap()

    with tile.TileContext(nc) as tc, ExitStack() as ctx:
        const = ctx.enter_context(tc.tile_pool(name="const", bufs=1))
        wp = ctx.enter_context(tc.tile_pool(name="wp", bufs=4))
        act = ctx.enter_context(tc.tile_pool(name="act", bufs=1))
        ev = ctx.enter_context(tc.tile_pool(name="ev", bufs=8))
        ps = ctx.enter_context(tc.tile_pool(name="ps", bufs=6, space="PSUM"))
        psp = ctx.enter_context(tc.tile_pool(name="psp", bufs=1, space="PSUM"))

        # weight DMAs round-robin over three HWDGE queues so supply isn't
        # capped by a single queue's descriptor rate
        dma_engs = [nc.sync, nc.scalar]
        rr = [0]

        def wdma(dst, src):
            dma_engs[rr[0] % len(dma_engs)].dma_start(dst, src)
            rr[0] += 1

        # Pre-warm the PE during the initial DMA wait: ~5 us of dummy
        # matmuls lift the HAM clock gate to 8/8 before real work arrives.
        warm = const.tile([P, P], dt.bfloat16, tag="warm")
        nc.vector.memset(warm[:], 0.0)
        warm_ps = psp.tile([P, P], dt.float32, tag="warmps")
        for _ in range(48):
            nc.tensor.matmul(warm_ps[:], warm[:], warm[:], start=True, stop=True)

        # inputs + L0 weights first, in small chunks split across both HWDGE
        # queues in first-use order so the PE can start within a few us;
        # other small constants go on the gpsimd SWDGE queue
        x_sb = {}
        w0_sb = {}
        chunk = GR[0] * KT[0] * P
        for t in ("o", "d"):
            x_sb[t] = const.tile([P, 2, BPC], dt.bfloat16,
                                 tag=f"x{t}", name=f"x{t}")
            wdma(x_sb[t][:], x_d[t].rearrange("p (k n) -> p k n", n=BPC))
            w0_sb[t] = const.tile([P, (MT[0] // GR[0]) * chunk], dt.bfloat16,
                                  tag=f"w0{t}", name=f"w0{t}")
            for c in range(MT[0] // GR[0]):
                wdma(w0_sb[t][:, c * chunk:(c + 1) * chunk], w_d[t, 0][c])
        bias_sb = {}
        for t in ("o", "d"):
            for l in range(4):
                bias_sb[t, l] = const.tile([P, MT[l]], dt.float32,
                                           tag=f"b{t}{l}", name=f"b{t}{l}")
                nc.gpsimd.dma_start(bias_sb[t, l][:], b_d[t, l][:])
        masks_sb = const.tile([P, A * A], dt.bfloat16, tag="masks")
        nc.gpsimd.dma_start(masks_sb[:], masks_d[:])

        # ---- Trunks: layers 0..2 with ReLU, feature-major throughout.
        # The two trunks are interleaved layer-by-layer so the PE has twice
        # the work per phase start, covering the weight-stream warm-up.
        cur = dict(x_sb)
        for l in range(3):
            for t in ("o", "d"):
                out_t = act.tile([P, MT[l], BPC], dt.bfloat16,
                                 tag=f"h{t}{l % 2}", name=f"h{t}{l}")
                for mtg in range(MT[l] // GR[l]):
                    if l == 0:
                        wt = w0_sb[t]
                    else:
                        wt = wp.tile([P, GR[l] * KT[l] * P], dt.bfloat16,
                                     tag="wbig")
                        wdma(wt[:], w_d[t, l][mtg])
                    for g in range(GR[l]):
                        mt = mtg * GR[l] + g
                        wcol = mt if l == 0 else g  # L0 tile is fully resident
                        pt = ps.tile([P, BPC], dt.float32, tag="mm")
                        for j in range(KT[l]):
                            nc.tensor.matmul(
                                pt[:], wt[:, ts(wcol * KT[l] + j, P)],
                                cur[t][:, j, :],
                                start=(j == 0), stop=(j == KT[l] - 1),
                            )
                        nc.scalar.activation(
                            out_t[:, mt, :], pt[:], AF.Relu,
                            bias=bias_sb[t, l][:, mt:mt + 1],
                        )
                cur[t] = out_t
        h = cur

        # ---- Layer 3 + bilinear diagonal, fused per 128-feature tile.
        pred_ps = psp.tile([A, BPC], dt.float32, tag="pred")
        for a in range(A):  # one weight DMA per trunk covers the whole action
            z_acc = ev.tile([P, BPC], dt.bfloat16, tag="zacc")
            wt = {}
            for t in ("o", "d"):
                wt[t] = wp.tile([P, GR[3] * KT[3] * P], dt.bfloat16,
                                tag="wbig", name=f"w3{t}")
                wdma(wt[t][:], w_d[t, 3][a])
            for g in range(GR[3]):
                mt = a * 4 + g
                s = {}
                for t in ("o", "d"):
                    pt = ps.tile([P, BPC], dt.float32, tag="mm")
                    for j in range(KT[3]):
                        nc.tensor.matmul(
                            pt[:], wt[t][:, ts(g * KT[3] + j, P)],
                            h[t][:, j, :],
                            start=(j == 0), stop=(j == KT[3] - 1),
                        )
                    s[t] = ev.tile([P, BPC], dt.bfloat16, tag="evict",
                                   name=f"s{t}")
                    nc.scalar.activation(
                        s[t][:], pt[:], AF.Identity,
                        bias=bias_sb[t, 3][:, mt:mt + 1],
                    )
                if g == 0:
                    nc.vector.tensor_mul(z_acc[:], s["o"][:], s["d"][:])
                else:
                    zt = ev.tile([P, BPC], dt.bfloat16, tag="ztmp")
                    nc.vector.tensor_mul(zt[:], s["o"][:], s["d"][:])
                    nc.vector.tensor_add(z_acc[:], z_acc[:], zt[:])
            nc.tensor.matmul(
                pred_ps[:], masks_sb[:, ts(a, A)], z_acc[:],
                start=(a == 0), stop=(a == A - 1),
            )

        pred_sb = ev.tile([A, BPC], dt.float32, tag="predsb")
        nc.vector.tensor_copy(pred_sb[:], pred_ps[:])
        nc.sync.dma_start(pred_d[:], pred_sb[:])

    nc.compile()
    return nc


def _prep_inputs(inputs):
    """Host-side layout/dtype prep shared across cores + per-core slices."""
    shared = {}

    for t, pfx in (("o", "obs"), ("d", "dlt")):
        for l in range(4):
            w = np.asarray(inputs[f"{pfx}_W{l}"], np.float32)
            b = np.asarray(inputs[f"{pfx}_b{l}"], np.float32)
            if t == "d" and l == 3:
                # permute columns (f,a) -> (a,f) to match obs layout
                w = w.reshape(H, F, A).transpose(0, 2, 1).reshape(H, DOUT)
                b = b.reshape(F, A).T.reshape(DOUT)
            shared[f"{t}w{l}"] = _tile_weight(w, GR[l])
            shared[f"{t}b{l}"] = _tile_bias(b)

    masks = np.zeros((P, A, A), np.float32)
    for a in range(A):
        masks[:, a, a] = 1.0
    shared["masks"] = np.ascontiguousarray(masks.reshape(P, A * A).astype(BF16))

    obsT = np.asarray(inputs["obs"], np.float32).T.astype(BF16)    # [256, 4096]
    dltT = np.asarray(inputs["deltas"], np.float32).T.astype(BF16)

    in_maps = []
    for c in range(NCORES):
        sl = slice(c * BPC, (c + 1) * BPC)
        m = dict(shared)
        m["xo"] = np.ascontiguousarray(
            obsT[:, sl].reshape(2, P, BPC).transpose(1, 0, 2).reshape(P, 2 * BPC))
        m["xd"] = np.ascontiguousarray(
            dltT[:, sl].reshape(2, P, BPC).transpose(1, 0, 2).reshape(P, 2 * BPC))
        in_maps.append(m)
    return in_maps


_PROGRAM = None


def kernel(**inputs):
    global _PROGRAM, LAST_RESULTS
    from concourse.bass_utils import run_bass_kernel_spmd

    if _PROGRAM is None:
        _PROGRAM = _build_program()
    in_maps = _prep_inputs(inputs)
    res = run_bass_kernel_spmd(_PROGRAM, in_maps, list(range(NCORES)))
    LAST_RESULTS = res
    out = np.empty((B, A), np.float32)
    for c in range(NCORES):
        out[c * BPC:(c + 1) * BPC] = res.results[c]["pred"].T
    return out


# revision 24
# speedup vs baseline: 1.0039x; 1.0011x over previous
"""Trainium2 Bass kernel for nn_BilinearPolicy (dense_mlp).

Math (reference):
  ob = trunk_obs(obs)      : [B,256] -> 2048 -> 2048 -> 2048 -> 16384 (ReLU between)
  dl = trunk_dlt(deltas)   : same shapes, different weights
  pred[b,a] = sum_f ob[b, a*512+f] * dl[b, f*32+a]            : [B, 32]

Strategy:
  * Data-parallel over batch: 8 cores x 512 rows, zero collectives.
  * Feature-major activations on chip ([feat(part), batch(free)]), so the
    torch-layout weights [din, dout] are used directly as matmul lhsT tiles
    and no transposes are ever needed. Inputs are transposed on host.
  * bf16 matmuls with fp32 PSUM accumulation. Biases applied during the
    mandatory PSUM->SBUF eviction on the Scalar engine (Relu / Identity).
  * dl's last-layer weight columns are permuted on host from (f,a) to (a,f)
    ordering, so the bilinear diagonal becomes: elementwise multiply of the
    two [16384, 512] outputs, then a segmented 512-row partition reduction.
    The 4 z-tiles of an action are accumulated on the DVE; one one-hot mask
    matmul per action accumulates pred^T [32, 512] in a single PSUM tile.
  * Weight streaming is the supply-critical path (~150 GB/s on one HWDGE
    queue with small lines): weights are pre-grouped on host so each DMA
    moves 4 m-tiles (2 MB, 16 KB/partition lines) and DMAs round-robin
    over the sync + scalar HWDGE queues. Small constants go via gpsimd
    SWDGE so they never queue ahead of weights.
"""

from contextlib import ExitStack

import numpy as np
import ml_dtypes

B, OBS, H, F, A = 4096, 256, 2048, 512, 32
DOUT = F * A            # 16384
NCORES = 8
BPC = B // NCORES       # 512 batch rows per core
P = 128

KT = [2, 16, 16, 16]    # k-tiles per layer
MT = [16, 16, 16, 128]  # m-tiles per layer
GR = [8, 4, 4, 4]       # m-tiles grouped per weight DMA

BF16 = ml_dtypes.bfloat16

# Filled with the BassKernelResults of the most recent run (for test harness).
LAST_RESULTS = None


def _tile_weight(w, G):
    """[D1, D2] fp32 -> [D2/(128G), 128(k), G*D1] bf16. Slice
    [:, (g*Kt + j)*128 : +128] of group tile mtg is the lhsT for
    k-tile j of m-tile mtg*G+g; every partition line is contiguous."""
    d1, d2 = w.shape
    kt, mt = d1 // P, d2 // P
    wt = w.reshape(kt, P, mt, P).transpose(2, 1, 0, 3)      # [mt, k, j, m]
    wt = wt.reshape(mt // G, G, P, kt * P).transpose(0, 2, 1, 3)
    wt = wt.reshape(mt // G, P, G * kt * P)
    return np.ascontiguousarray(wt.astype(BF16))


def _tile_bias(b):
    """[D2] fp32 -> [128, D2/128] fp32; column mt holds bias for m-tile mt
    as a per-partition scalar."""
    return np.ascontiguousarray(b.reshape(-1, P).T.astype(np.float32))


def _build_program():
    import concourse.bass as bass
    import concourse.tile as tile
    from concourse import bacc, mybir
    from concourse.bass import ts

    dt = mybir.dt
    AF = mybir.ActivationFunctionType

    nc = bacc.Bacc(
        "TRN2",
        target_bir_lowering=False,
        debug=False,
        enable_asserts=True,
        num_devices=NCORES,
    )

    def din(name, shape, dtype):
        return nc.dram_tensor(name, shape, dtype, kind="ExternalInput").ap()

    x_d = {
        "o": din("xo", [P, 2 * BPC], dt.bfloat16),
        "d": din("xd", [P, 2 * BPC], dt.bfloat16),
    }
    w_d = {}
    b_d = {}
    for t in ("o", "d"):
        for l in range(4):
            w_d[t, l] = din(f"{t}w{l}", [MT[l] // GR[l], P, GR[l] * KT[l] * P],
                            dt.bfloat16)
            b_d[t, l] = din(f"{t}b{l}", [P, MT[l]], dt.float32)
    masks_d = din("masks", [P, A * A], dt.bfloat16)
    pred_d = nc.dram_tensor("pred", [A, BPC], dt.float32, kind="ExternalOutput").ap()

    with tile.TileContext(nc) as tc, ExitStack() as ctx:
        const = ctx.enter_context(tc.tile_pool(name="const", bufs=1))
        wp = ctx.enter_context(tc.tile_pool(name="wp", bufs=4))
        act = ctx.enter_context(tc.tile_pool(name="act", bufs=1))
        ev = ctx.enter_context(tc.tile_pool(name="ev", bufs=8))
        ps = ctx.enter_context(tc.tile_pool(name="ps", bufs=6, space="PSUM"))
        psp = ctx.enter_context(tc.tile_pool(name="psp", bufs=1, space="PSUM"))

        # weight DMAs round-robin over three HWDGE queues so supply isn't
        # capped by a single queue's descriptor rate
        dma_engs = [nc.sync, nc.scalar]
        rr = [0]

        def wdma(dst, src):
            dma_engs[rr[0] % len(dma_engs)].dma_start(dst, src)
            rr[0] += 1

        # inputs + L0 weights first, in two chunks split across both HWDGE
        # queues in first-use order so the PE can start within a few us;
        # other small constants go on the gpsimd SWDGE queue
        x_sb = {}
        w0_sb = {}
        chunk = GR[0] * KT[0] * P
        for t in ("o", "d"):
            x_sb[t] = const.tile([P, 2, BPC], dt.bfloat16,
                                 tag=f"x{t}", name=f"x{t}")
            wdma(x_sb[t][:], x_d[t].rearrange("p (k n) -> p k n", n=BPC))
            w0_sb[t] = const.tile([P, (MT[0] // GR[0]) * chunk], dt.bfloat16,
                                  tag=f"w0{t}", name=f"w0{t}")
            for c in range(MT[0] // GR[0]):
                wdma(w0_sb[t][:, c * chunk:(c + 1) * chunk], w_d[t, 0][c])
        bias_sb = {}
        for t in ("o", "d"):
            for l in range(4):
                bias_sb[t, l] = const.tile([P, MT[l]], dt.float32,
                                           tag=f"b{t}{l}", name=f"b{t}{l}")
                nc.gpsimd.dma_start(bias_sb[t, l][:], b_d[t, l][:])
        masks_sb = const.tile([P, A * A], dt.bfloat16, tag="masks")
        nc.gpsimd.dma_start(masks_sb[:], masks_d[:])

        # ---- Trunks: layers 0..2 with ReLU, feature-major throughout.
        # The two trunks are interleaved layer-by-layer so the PE has twice
        # the work per phase start, covering the weight-stream warm-up.
        cur = dict(x_sb)
        for l in range(3):
            for t in ("o", "d"):
                out_t = act.tile([P, MT[l], BPC], dt.bfloat16,
                                 tag=f"h{t}{l % 2}", name=f"h{t}{l}")
                for mtg in range(MT[l] // GR[l]):
                    if l == 0:
                        wt = w0_sb[t]
                    else:
                        wt = wp.tile([P, GR[l] * KT[l] * P], dt.bfloat16,
                                     tag="wbig")
                        wdma(wt[:], w_d[t, l][mtg])
                    for g in range(GR[l]):
                        mt = mtg * GR[l] + g
                        wcol = mt if l == 0 else g  # L0 tile is fully resident
                        pt = ps.tile([P, BPC], dt.float32, tag="mm")
                        for j in range(KT[l]):
                            nc.tensor.matmul(
                                pt[:], wt[:, ts(wcol * KT[l] + j, P)],
                                cur[t][:, j, :],
                                start=(j == 0), stop=(j == KT[l] - 1),
                            )
                        nc.scalar.activation(
                            out_t[:, mt, :], pt[:], AF.Relu,
                            bias=bias_sb[t, l][:, mt:mt + 1],
                        )
                cur[t] = out_t
        h = cur

        # ---- Layer 3 + bilinear diagonal, fused per 128-feature tile.
        pred_ps = psp.tile([A, BPC], dt.float32, tag="pred")
        for a in range(A):  # one weight DMA per trunk covers the whole action
            z_acc = ev.tile([P, BPC], dt.bfloat16, tag="zacc")
            wt = {}
            for t in ("o", "d"):
                wt[t] = wp.tile([P, GR[3] * KT[3] * P], dt.bfloat16,
                                tag="wbig", name=f"w3{t}")
                wdma(wt[t][:], w_d[t, 3][a])
            for g in range(GR[3]):
                mt = a * 4 + g
                s = {}
                for t in ("o", "d"):
                    pt = ps.tile([P, BPC], dt.float32, tag="mm")
                    for j in range(KT[3]):
                        nc.tensor.matmul(
                            pt[:], wt[t][:, ts(g * KT[3] + j, P)],
                            h[t][:, j, :],
                            start=(j == 0), stop=(j == KT[3] - 1),
                        )
                    s[t] = ev.tile([P, BPC], dt.bfloat16, tag="evict",
                                   name=f"s{t}")
                    nc.scalar.activation(
                        s[t][:], pt[:], AF.Identity,
                        bias=bias_sb[t, 3][:, mt:mt + 1],
                    )
                if g == 0:
                    nc.vector.tensor_mul(z_acc[:], s["o"][:], s["d"][:])
                else:
                    zt = ev.tile([P, BPC], dt.bfloat16, tag="ztmp")
                    nc.vector.tensor_mul(zt[:], s["o"][:], s["d"][:])
                    nc.vector.tensor_add(z_acc[:], z_acc[:], zt[:])
            nc.tensor.matmul(
                pred_ps[:], masks_sb[:, ts(a, A)], z_acc[:],
                start=(a == 0), stop=(a == A - 1),
            )

        pred_sb = ev.tile([A, BPC], dt.float32, tag="predsb")
        nc.vector.tensor_copy(pred_sb[:], pred_ps[:])
        nc.sync.dma_start(pred_d[:], pred_sb[:])

    nc.compile()
    return nc


def _prep_inputs(inputs):
    """Host-side layout/dtype prep shared across cores + per-core slices."""
    shared = {}

    for t, pfx in (("o", "obs"), ("d", "dlt")):
        for l in range(4):
            w = np.asarray(inputs[f"{pfx}_W{l}"], np.float32)
            b = np.asarray(inputs[f"{pfx}_b{l}"], np.float32)
            if t == "d" and l == 3:
                # permute columns (f,a) -> (a,f) to match obs layout
                w = w.reshape(H, F, A).transpose(0, 2, 1).reshape(H, DOUT)
                b = b.reshape(F, A).T.reshape(DOUT)
            shared[f"{t}w{l}"] = _tile_weight(w, GR[l])
            shared[f"{t}b{l}"] = _tile_bias(b)

    masks = np.zeros((P, A, A), np.float32)
    for a in range(A):
        masks[:, a, a] = 1.0
    shared["masks"] = np.ascontiguousarray(masks.reshape(P, A * A).astype(BF16))

    obsT = np.asarray(inputs["obs"], np.float32).T.astype(BF16)    # [256, 4096]
    dltT = np.asarray(inputs["deltas"], np.float32).T.astype(BF16)

    in_maps = []
    for c in range(NCORES):
        sl = slice(c * BPC, (c + 1) * BPC)
        m = dict(shared)
        m["xo"] = np.ascontiguousarray(
            obsT[:, sl].reshape(2, P, BPC).transpose(1, 0, 2).reshape(P, 2 * BPC))
        m["xd"] = np.ascontiguousarray(
            dltT[:, sl].reshape(2, P, BPC).transpose(1, 0, 2).reshape(P, 2 * BPC))
        in_maps.append(m)
    return in_maps


_PROGRAM = None


def kernel(**inputs):
    global _PROGRAM, LAST_RESULTS
    from concourse.bass_utils import run_bass_kernel_spmd

    if _PROGRAM is None:
        _PROGRAM = _build_program()
    in_maps = _prep_inputs(inputs)
    res = run_bass_kernel_spmd(_PROGRAM, in_maps, list(range(NCORES)))
    LAST_RESULTS = res
    out = np.empty((B, A), np.float32)
    for c in range(NCORES):
        out[c * BPC:(c + 1) * BPC] = res.results[c]["pred"].T
    return out
